# revision 20
# baseline (speedup 1.0000x reference)
"""Trainium2 Bass kernel: GAT-style attention layer, data-parallel over 8 NeuronCores.

Reference computation (per node n, K=32 neighbors, D=128 features, L=64 labels):
    h     = lrelu(x @ W)                  [N,K,D]
    e     = lrelu(h @ v + bias)           [N,K,1]
    alpha = softmax_k(e)                  [N,K]
    out   = sum_k alpha[n,k] * labels[n,k,:]   [N,L]

Sharding: pure data parallel over nodes (6250/core, zero-padded to 6400).

Structure (v3): every contraction over a 128-partition axis rides the PE with
the LARGE tensor as the *stationary* operand and a 1..4-column moving operand
(PE engine time ~ moving columns only):
  mm1    z^T[e,(k,n)] = W^T @ x^T      16x512-col fp8 matmuls / 256-node tile
  act    h = lrelu(z) (ScalarE chunks, full Prelu) or relu(z) (VectorE
         chunks; the 0.2z part is restored by an x-correction matmul)
  score  s[n, 32s+k]: per (k,sub) a 1-column matmul with the h slice as
         stationary (+ 0.2(Wv)^T x correction for VectorE chunks)
  e/exp  ACT Prelu + Exp on [128,64]
  alphaT 8 32x32 PE transposes place node-quarter j's exp-weights at
         partition block 32j of a persistent (startup-zeroed) PSUM tile;
         one DVE copy -> aT4 [128, 256n] with zeros off-block
  agg    out^T[l, 4 nodes] per matmul: stationary = 4 nodes' label blocks
         stacked [128=(4x32k), 64l] fp8; off-block zeros in aT4 kill the
         cross-node terms. 64 Ldweights+matmuls per tile.
  sums   gpsimd partition-reduce of aT4 -> [1, 256]; softmax 1/sum applied
         on the HOST (sums are DMA'd out, 1KB/tile)

Quantization: x fp8-e3m4, labels fp8-e3m4, weights bf16, out bf16.
Per-core DRAM traffic ~39MB; DMA floor ~4.5us per 256-node tile.
"""
import sys

sys.path.insert(0, "/opt/trn_rl_repo")
import numpy as np

N, K, D, L = 50000, 32, 128, 64
NEG = 0.2
NCORES = 8
NPER = N // NCORES          # 6250
TN = 256                    # nodes per tile
NSUB = TN // 128            # sub-tiles of 128 nodes
NPAD = 6400                 # padded nodes per core
NT = NPAD // TN             # 25 tiles
NCH = 8                     # mm1 chunks per tile (1024 cols each)

# relu-chunk engine schedule: 'A' = ScalarE (full Prelu), 'D' = VectorE
# (relu-only + PE x-correction). Chunk 0 must be 'D' so ACT starts each tile
# with the previous tile's Prelu/Exp.
SCHED = "DADADADA"

LAST_RESULT = None
_cache = {}


def build(with_bias, debug=False):
    import concourse.bass as bass
    import concourse.tile as tile
    from concourse import bacc, mybir

    f32 = mybir.dt.float32
    bf16 = mybir.dt.bfloat16
    f8 = mybir.dt.float8e3
    AF = mybir.ActivationFunctionType
    OP = mybir.AluOpType
    PSUM = bass.MemorySpace.PSUM
    nt = NT

    nc = bacc.Bacc(
        "TRN2", target_bir_lowering=False, debug=False, num_devices=NCORES
    )
    x_ext = nc.declare_dram_parameter("x", [nt - 1, 128, K * TN], f8, False)
    lab_ext = nc.declare_dram_parameter("lab", [nt - 1, 128, 64 * 65], f8, False)
    # last tile holds only 128 valid node slots: packed layout col = k*128+n
    x24_ext = nc.declare_dram_parameter("x24", [128, K * 128], f8, False)
    lab24_ext = nc.declare_dram_parameter("lab24", [128, 32 * 65], f8, False)
    # W | v10 | v08 | wv02 packed in one tensor -> one startup DMA (each
    # separate small DMA costs ~650ns of serial HWDGE time)
    wpk_ext = nc.declare_dram_parameter("wpk", [D, D + 3], bf16, False)
    b64_ext = nc.declare_dram_parameter("b64", [128, 64], f32, False)
    out_ext = nc.declare_dram_parameter("out", [nt, L + 1, TN], bf16, isOutput=True)
    if debug:
        sdump_ext = nc.declare_dram_parameter("sdump", [nt, 128, 64], f32, isOutput=True)
        wdump_ext = nc.declare_dram_parameter("wdump", [nt, 128, 64], bf16, isOutput=True)
        hdump_ext = nc.declare_dram_parameter("hdump", [nt, 128, 2048], bf16, isOutput=True)

    with tile.TileContext(nc) as tc:
        with (
            tc.tile_pool(name="const", bufs=1) as const,
            tc.tile_pool(name="xp", bufs=3) as xp,
            tc.tile_pool(name="labp", bufs=2) as labp,
            tc.tile_pool(name="hp", bufs=2) as hp,
            tc.tile_pool(name="smallp", bufs=2) as smallp,
            tc.tile_pool(name="outp", bufs=2) as outp,
            tc.tile_pool(name="zps", bufs=3, space=PSUM) as zps,
            tc.tile_pool(name="smps", bufs=1, space=PSUM) as smps,
        ):
            wpk = const.tile([128, D + 3], bf16)
            nc.sync.dma_start(wpk[:], wpk_ext[:])
            W_sb = wpk[:, 0:128]
            v10 = wpk[:, 128:129]
            v08 = wpk[:, 129:130]
            wv02 = wpk[:, 130:131]
            if with_bias:
                b64 = const.tile([128, 64], f32)
                nc.sync.dma_start(b64[:], b64_ext[:])
            ones = const.tile([128, 128], bf16)
            nc.vector.memset(ones[:], 1.0)
            ident = const.tile([128, 128], bf16)         # identity matrix
            nc.gpsimd.affine_select(
                ident[:], ones[:], pattern=[[1, 128]],
                compare_op=OP.is_equal, fill=0.0, base=0, channel_multiplier=-1,
            )
            # One PSUM bank holds all the small tiles, manually carved:
            # cols 0:64 / 64:128 = s_ps (alternating per tile), 128:384 =
            # o_ps [65,256], 384:512 bitcast bf16 = aT4_ps [128,256].
            smalls = smps.tile([128, 512], f32, name="smalls", tag="smalls")
            s_ps_ab = (smalls[:, 0:64], smalls[:, 64:128])
            o_ps_ap = smalls[0:65, 128:384]
            aT4_ps = smalls[:, 384:512].bitcast(bf16)
            # persistent exp-weight transpose target: node-quarter j occupies
            # partition block 32j; everything off-block is zeroed ONCE here
            # and never written again, so cross-node terms in the batched agg
            # matmul multiply against exact zeros. (memset the underlying f32
            # slice: all-zero bits read back as bf16 zeros. A memset or copy
            # through the bf16 bitcast view fails walrus codegen.)
            nc.vector.memset(smalls[:, 384:512], 0.0)

            # PE warmup burst: dummy matmuls (deps ready ~1us in) while the
            # first x tile loads; the p-state model needs ~3.4us of early PE
            # activity to reach the full 2.4GHz clock.
            warm_ps = zps.tile([128, 1024], f32, name="warm_ps", tag="z")
            for _ in range(32):
                nc.tensor.matmul(
                    warm_ps[:, 0:128], W_sb, ones[:], skip_group_check=True
                )

            prev = None   # state of tile t-1 awaiting its score/agg tail

            def emit_head(st):
                """e = lrelu(s) (+bias), w = exp(e). First ACT ops of a tile."""
                ns = st["subs"]
                s_ps = st["s_ps"][:, 0:32 * ns]
                e_sb = smallp.tile([128, 64], f32, name="e_sb", tag="e_sb")
                if with_bias:
                    sb = smallp.tile([128, 64], f32, name="sb", tag="sb")
                    nc.vector.tensor_tensor(sb[:, 0:32 * ns], s_ps, b64[:, 0:32 * ns], OP.add)
                    nc.scalar.activation(e_sb[:, 0:32 * ns], sb[:, 0:32 * ns], AF.Prelu, alpha=NEG)
                else:
                    nc.scalar.activation(e_sb[:, 0:32 * ns], s_ps, AF.Prelu, alpha=NEG)
                w_sb = smallp.tile([128, 64], bf16, name="w_sb", tag="w_sb")
                nc.scalar.activation(w_sb[:, 0:32 * ns], e_sb[:, 0:32 * ns], AF.Exp)
                st["w_sb"] = w_sb
                if debug:
                    sd = smallp.tile([128, 64], f32, name="sd", tag="sd")
                    nc.vector.tensor_copy(sd[:, 0:32 * ns], s_ps)
                    nc.sync.dma_start(sdump_ext[st["t"]][:, 0:32 * ns], sd[:, 0:32 * ns])
                    nc.sync.dma_start(wdump_ext[st["t"]][:, 0:32 * ns], w_sb[:, 0:32 * ns])
                    nc.sync.dma_start(hdump_ext[st["t"]], st["h01"])

            def emit_transp(st):
                """8 32x32 transposes: sub s node-quarter j -> aT4_ps rows
                [32j,32j+32), cols [128s+32j, +32). Then one DVE copy to SBUF
                (zeros off-block come along) + gpsimd per-node sums + DMA."""
                w_sb = st["w_sb"]
                for s in range(st["subs"]):
                    for j in range(4):
                        nc.tensor.transpose(
                            aT4_ps[32 * j:32 * j + 32,
                                    128 * s + 32 * j:128 * s + 32 * j + 32],
                            w_sb[32 * j:32 * j + 32, 32 * s:32 * s + 32],
                            ident[32 * j:32 * j + 32, 32 * j:32 * j + 32],
                            tile_position=(32 * j, 32 * j),
                        )
                aT4 = smallp.tile([128, TN], bf16, name="aT4", tag="aT4")
                nc.vector.tensor_copy(
                    aT4[:, 0:128 * st["subs"]], aT4_ps[:, 0:128 * st["subs"]]
                )
                st["aT4"] = aT4
                st["o_ps"] = o_ps_ap

            def emit_agg(st, q):
                """Aggregation for node groups [16q, 16q+16): one 4-column
                matmul per group of 4 nodes; stationary = their label blocks
                stacked [128, 64] fp8 + a 65th all-ones column whose output
                row is the per-node exp-weight sum (softmax denominator)."""
                lab_sb, aT4, o_ps = st["lab_sb"], st["aT4"], st["o_ps"]
                gmax = 32 * st["subs"]
                for g in range(16 * q, min(16 * q + 16, gmax)):
                    # group g = (s, m): the 4 nodes 128s+32j+m (j=0..3), one
                    # per partition block of aT4 -> moving cols stride 32.
                    # Output columns are contiguous [4g, 4g+4); the host
                    # remaps column 128s+4m+j back to node 128s+32j+m.
                    s_g, m = g // 32, g % 32
                    nc.tensor.matmul(
                        o_ps[:, 4 * g:4 * g + 4],
                        lab_sb[:, 65 * g:65 * g + 65],
                        aT4[:, 128 * s_g + m:128 * s_g + m + 97:32],
                    )

            def emit_out(st):
                w = 128 * st["subs"]
                o_sb = outp.tile([L + 1, TN], bf16, name="o_sb", tag="o_sb")
                nc.scalar.activation(o_sb[:, 0:w], st["o_ps"][:, 0:w], AF.Copy)
                nc.sync.dma_start(out_ext[st["t"]][:, 0:w], o_sb[:, 0:w])

            for t in range(nt):
                last = t == nt - 1
                tsubs = 1 if last else NSUB
                stride = 128 if last else TN      # nodes per k-slice
                ncht = 4 if last else NCH         # 1024-col mm1 chunks
                kpc = 1024 // stride              # k values per chunk
                sched = (SCHED[0] + SCHED[1]) * 2 if last else SCHED
                x_sb = xp.tile([128, K * TN], f8)
                if t == 0:
                    # quarter the first x load so chunk 0's matmuls start
                    # after ~256KB instead of a full 1MB
                    qn = K * TN // 4
                    for qi in range(4):
                        nc.sync.dma_start(
                            x_sb[:, qi * qn:(qi + 1) * qn],
                            x_ext[t][:, qi * qn:(qi + 1) * qn],
                        )
                elif last:
                    nc.sync.dma_start(x_sb[:, 0:K * 128], x24_ext[:])
                else:
                    nc.sync.dma_start(x_sb[:], x_ext[t][:])
                lab_sb = labp.tile([128, 64 * 65], f8)
                if last:
                    nc.sync.dma_start(lab_sb[:, 0:32 * 65], lab24_ext[:])
                else:
                    nc.sync.dma_start(lab_sb[:], lab_ext[t][:])

                h_sb = hp.tile([128, K * TN], bf16)
                s_ps = s_ps_ab[t % 2]

                def emit_scores(c):
                    # score columns for chunk c's k values (ready once relu c
                    # done). For relu-only (DVE) chunks each column is a
                    # 2-matmul group: h-term then the 0.2(Wv)^T x correction.
                    # Keeping the group members ADJACENT matters: a group
                    # split across other matmuls loses the first term.
                    dve = sched[c] == "D"
                    for k in range(kpc * c, kpc * (c + 1)):
                        for s in range(tsubs):
                            col = 32 * s + k
                            base = k * stride + s * 128
                            nc.tensor.matmul(
                                s_ps[:, col:col + 1],
                                h_sb[:, base:base + 128],
                                v08 if dve else v10,
                                start=True, stop=not dve,
                            )
                            if dve:
                                nc.tensor.matmul(
                                    s_ps[:, col:col + 1],
                                    x_sb[:, base:base + 128], wv02,
                                    start=False, stop=True,
                                )

                for c in range(ncht):
                    z_ps = zps.tile([128, 1024], f32, name="z_ps", tag="z")
                    nc.tensor.matmul(
                        z_ps[:, 0:512], W_sb, x_sb[:, c * 1024:c * 1024 + 512]
                    )
                    nc.tensor.matmul(
                        z_ps[:, 512:1024], W_sb,
                        x_sb[:, c * 1024 + 512:(c + 1) * 1024],
                    )
                    zv = z_ps[:]
                    hv = h_sb[:, c * 1024:(c + 1) * 1024]
                    if sched[c] == "A":
                        nc.scalar.activation(hv, zv, AF.Prelu, alpha=NEG)
                    else:
                        nc.vector.tensor_scalar_max(hv, zv, 0.0)
                    if c == 0 and prev is not None:
                        # the head chain gates next-tile aT4/agg: make it
                        # look one tile older so the scheduler runs it ASAP
                        with tc.high_priority(offset=600):
                            emit_head(prev)
                    if c == 1 and prev is not None:
                        with tc.high_priority(offset=600):
                            emit_transp(prev)
                    if prev is not None:
                        if last:
                            if c in (2, 3):
                                emit_agg(prev, 2 * (c - 2))
                                emit_agg(prev, 2 * (c - 2) + 1)
                        elif c in (3, 4, 5, 6):
                            emit_agg(prev, c - 3)
                    if c >= 3:
                        emit_scores(c - 3)
                    if not last and c == 7 and prev is not None:
                        emit_out(prev)
                if last and prev is not None:
                    emit_out(prev)
                for cc in range(max(ncht - 3, 0), ncht):
                    emit_scores(cc)

                prev = {"t": t, "s_ps": s_ps, "lab_sb": lab_sb, "subs": tsubs,
                        "h01": h_sb[:, 0:2048]}

            # drain the last tile
            emit_head(prev)
            emit_transp(prev)
            for q in range(4):
                emit_agg(prev, q)
            emit_out(prev)
    nc.compile()
    return nc


def shard_x(x, nt=NT, nper=NPER):
    import ml_dtypes

    f8 = ml_dtypes.float8_e3m4
    xs = np.zeros((nt * TN, K, D), f8)
    xs[:nper] = x.astype(f8)
    # [t, n, k, d] -> [t, d, k, n] -> col = k*TN + n  (tiles 0..nt-2)
    xf = np.ascontiguousarray(
        xs[:(nt - 1) * TN].reshape(nt - 1, TN, K, D).transpose(0, 3, 2, 1)
    ).reshape(nt - 1, 128, K * TN)
    # last tile: first 128 node slots only, packed col = k*128 + n
    x24 = np.ascontiguousarray(
        xs[(nt - 1) * TN:(nt - 1) * TN + 128].transpose(2, 1, 0)
    ).reshape(128, K * 128)
    return xf, x24


def shard_lab(lab, nt=NT, nper=NPER):
    import ml_dtypes

    f8 = ml_dtypes.float8_e3m4
    ls = np.zeros((nt * TN, K, L + 1), f8)
    ls[:nper, :, :L] = lab.astype(f8)
    ls[:, :, L] = f8(1.0)   # ones column -> per-node exp-weight sums
    # node n = 128s + 32j + m belongs to group g = 32s + m with quarter j:
    # its labels sit at rows 32j + k of group block [65g, 65g+65)
    l6 = ls[:(nt - 1) * TN].reshape(nt - 1, NSUB, 4, 32, K, L + 1)
    lf = np.ascontiguousarray(
        l6.transpose(0, 2, 4, 1, 3, 5)           # [t, j, k, s, m, l]
    ).reshape(nt - 1, 128, 64 * 65)
    l24 = ls[(nt - 1) * TN:(nt - 1) * TN + 128].reshape(4, 32, K, L + 1)
    lab24 = np.ascontiguousarray(
        l24.transpose(0, 2, 1, 3)                # [j, k, m, l]
    ).reshape(128, 32 * 65)
    return lf, lab24


def make_in_maps(inputs):
    import ml_dtypes

    bf16 = ml_dtypes.bfloat16
    x = np.asarray(inputs["para_neighbors"], np.float32)
    lab = np.asarray(inputs["para_nei_labels"], np.float32)
    Wm = np.ascontiguousarray(np.asarray(inputs["linear"], np.float32))
    v = np.ascontiguousarray(np.asarray(inputs["e_vec"], np.float32))
    b = np.asarray(inputs["bias"], np.float32).reshape(K)

    Wb = Wm.astype(bf16).astype(np.float32)
    vb = v.astype(bf16).astype(np.float32)
    wpk = np.zeros((128, 131), bf16)
    wpk[:, 0:128] = Wm.astype(bf16)
    wpk[:, 128] = vb.astype(bf16).reshape(128)
    wpk[:, 129] = (0.8 * vb).astype(bf16).reshape(128)
    # 0.2*(W@v) from the bf16-rounded W/v so the correction matches the PE's z
    wpk[:, 130] = (NEG * (Wb @ vb)).astype(bf16).reshape(128)
    wpk = np.ascontiguousarray(wpk)
    # b64[p, 32s+k] = bias[k] (same for every partition row)
    b64 = np.ascontiguousarray(
        np.tile(np.concatenate([b, b])[None, :], (128, 1))
    ).astype(np.float32)

    in_maps = []
    for i in range(NCORES):
        xf, x24 = shard_x(x[i * NPER:(i + 1) * NPER])
        lf, lab24 = shard_lab(lab[i * NPER:(i + 1) * NPER])
        in_maps.append({"x": xf, "lab": lf, "x24": x24, "lab24": lab24,
                        "wpk": wpk, "b64": b64})
    return in_maps


def unshard_output(res_i):
    # out[t, l, c]: column c = 128s + 4m + j holds node n = 128s + 32j + m
    # of tile t; row L = exp-weight sum. Softmax normalization happens here.
    o = np.asarray(res_i["out"]).astype(np.float32)      # [nt, L+1, TN]
    c = np.arange(TN)
    node_of_c = 128 * (c // 128) + 32 * (c % 4) + (c % 128) // 4
    inv = np.empty(TN, np.int64)
    inv[node_of_c] = c
    o = o[:, :, inv]                                     # column c' = node c'
    raw = o[:, :L].transpose(0, 2, 1).reshape(NT * TN, L)
    s = o[:, L].reshape(NT * TN)
    return (raw[:NPER] / s[:NPER, None]).astype(np.float32)


def kernel(para_neighbors, para_nei_labels, linear, e_vec, bias):
    from concourse.bass_utils import run_bass_kernel_spmd

    global LAST_RESULT
    with_bias = bool(np.any(np.asarray(bias)))
    key = ("nc", with_bias)
    if key not in _cache:
        _cache[key] = build(with_bias)
        _cache["nc"] = _cache[key]
    nc = _cache[key]

    in_maps = make_in_maps({
        "para_neighbors": para_neighbors, "para_nei_labels": para_nei_labels,
        "linear": linear, "e_vec": e_vec, "bias": bias,
    })
    res = run_bass_kernel_spmd(nc, in_maps, core_ids=list(range(NCORES)))
    LAST_RESULT = res
    outs = [unshard_output(res.results[i]) for i in range(NCORES)]
    return np.ascontiguousarray(np.concatenate(outs, axis=0))


# revision 21
# speedup vs baseline: 1.0010x; 1.0010x over previous
"""Trainium2 Bass kernel: GAT-style attention layer, data-parallel over 8 NeuronCores.

Reference computation (per node n, K=32 neighbors, D=128 features, L=64 labels):
    h     = lrelu(x @ W)                  [N,K,D]
    e     = lrelu(h @ v + bias)           [N,K,1]
    alpha = softmax_k(e)                  [N,K]
    out   = sum_k alpha[n,k] * labels[n,k,:]   [N,L]

Sharding: pure data parallel over nodes (6250/core, zero-padded to 6400).

Structure (v3): every contraction over a 128-partition axis rides the PE with
the LARGE tensor as the *stationary* operand and a 1..4-column moving operand
(PE engine time ~ moving columns only):
  mm1    z^T[e,(k,n)] = W^T @ x^T      16x512-col fp8 matmuls / 256-node tile
  act    h = lrelu(z) (ScalarE chunks, full Prelu) or relu(z) (VectorE
         chunks; the 0.2z part is restored by an x-correction matmul)
  score  s[n, 32s+k]: per (k,sub) a 1-column matmul with the h slice as
         stationary (+ 0.2(Wv)^T x correction for VectorE chunks)
  e/exp  ACT Prelu + Exp on [128,64]
  alphaT 8 32x32 PE transposes place node-quarter j's exp-weights at
         partition block 32j of a persistent (startup-zeroed) PSUM tile;
         one DVE copy -> aT4 [128, 256n] with zeros off-block
  agg    out^T[l, 4 nodes] per matmul: stationary = 4 nodes' label blocks
         stacked [128=(4x32k), 64l] fp8; off-block zeros in aT4 kill the
         cross-node terms. 64 Ldweights+matmuls per tile.
  sums   gpsimd partition-reduce of aT4 -> [1, 256]; softmax 1/sum applied
         on the HOST (sums are DMA'd out, 1KB/tile)

Quantization: x fp8-e3m4, labels fp8-e3m4, weights bf16, out bf16.
Per-core DRAM traffic ~39MB; DMA floor ~4.5us per 256-node tile.
"""
import sys

sys.path.insert(0, "/opt/trn_rl_repo")
import numpy as np

N, K, D, L = 50000, 32, 128, 64
NEG = 0.2
NCORES = 8
NPER = N // NCORES          # 6250
TN = 256                    # nodes per tile
NSUB = TN // 128            # sub-tiles of 128 nodes
NPAD = 6400                 # padded nodes per core
NT = NPAD // TN             # 25 tiles
NCH = 8                     # mm1 chunks per tile (1024 cols each)

# relu-chunk engine schedule: 'A' = ScalarE (full Prelu), 'D' = VectorE
# (relu-only + PE x-correction). Chunk 0 must be 'D' so ACT starts each tile
# with the previous tile's Prelu/Exp.
SCHED = "DADADADA"

LAST_RESULT = None
_cache = {}


def build(with_bias, debug=False):
    import concourse.bass as bass
    import concourse.tile as tile
    from concourse import bacc, mybir

    f32 = mybir.dt.float32
    bf16 = mybir.dt.bfloat16
    f8 = mybir.dt.float8e3
    AF = mybir.ActivationFunctionType
    OP = mybir.AluOpType
    PSUM = bass.MemorySpace.PSUM
    nt = NT

    nc = bacc.Bacc(
        "TRN2", target_bir_lowering=False, debug=False, num_devices=NCORES
    )
    x_ext = nc.declare_dram_parameter("x", [nt, 128, K * TN], f8, False)
    lab_ext = nc.declare_dram_parameter("lab", [nt, 128, 64 * 65], f8, False)
    # W | v10 | v08 | wv02 packed in one tensor -> one startup DMA (each
    # separate small DMA costs ~650ns of serial HWDGE time)
    wpk_ext = nc.declare_dram_parameter("wpk", [D, D + 3], bf16, False)
    b64_ext = nc.declare_dram_parameter("b64", [128, 64], f32, False)
    out_ext = nc.declare_dram_parameter("out", [nt, L + 1, TN], bf16, isOutput=True)
    if debug:
        sdump_ext = nc.declare_dram_parameter("sdump", [nt, 128, 64], f32, isOutput=True)
        wdump_ext = nc.declare_dram_parameter("wdump", [nt, 128, 64], bf16, isOutput=True)
        hdump_ext = nc.declare_dram_parameter("hdump", [nt, 128, 2048], bf16, isOutput=True)

    with tile.TileContext(nc) as tc:
        with (
            tc.tile_pool(name="const", bufs=1) as const,
            tc.tile_pool(name="xp", bufs=3) as xp,
            tc.tile_pool(name="labp", bufs=2) as labp,
            tc.tile_pool(name="hp", bufs=2) as hp,
            tc.tile_pool(name="smallp", bufs=2) as smallp,
            tc.tile_pool(name="outp", bufs=2) as outp,
            tc.tile_pool(name="zps", bufs=3, space=PSUM) as zps,
            tc.tile_pool(name="smps", bufs=1, space=PSUM) as smps,
        ):
            wpk = const.tile([128, D + 3], bf16)
            nc.sync.dma_start(wpk[:], wpk_ext[:])
            W_sb = wpk[:, 0:128]
            v10 = wpk[:, 128:129]
            v08 = wpk[:, 129:130]
            wv02 = wpk[:, 130:131]
            if with_bias:
                b64 = const.tile([128, 64], f32)
                nc.sync.dma_start(b64[:], b64_ext[:])
            ones = const.tile([128, 128], bf16)
            nc.vector.memset(ones[:], 1.0)
            ident = const.tile([128, 128], bf16)         # identity matrix
            nc.gpsimd.affine_select(
                ident[:], ones[:], pattern=[[1, 128]],
                compare_op=OP.is_equal, fill=0.0, base=0, channel_multiplier=-1,
            )
            # One PSUM bank holds all the small tiles, manually carved:
            # cols 0:64 / 64:128 = s_ps (alternating per tile), 128:384 =
            # o_ps [65,256], 384:512 bitcast bf16 = aT4_ps [128,256].
            smalls = smps.tile([128, 512], f32, name="smalls", tag="smalls")
            s_ps_ab = (smalls[:, 0:64], smalls[:, 64:128])
            o_ps_ap = smalls[0:65, 128:384]
            aT4_ps = smalls[:, 384:512].bitcast(bf16)
            # persistent exp-weight transpose target: node-quarter j occupies
            # partition block 32j; everything off-block is zeroed ONCE here
            # and never written again, so cross-node terms in the batched agg
            # matmul multiply against exact zeros. (memset the underlying f32
            # slice: all-zero bits read back as bf16 zeros. A memset or copy
            # through the bf16 bitcast view fails walrus codegen.)
            nc.vector.memset(smalls[:, 384:512], 0.0)

            # PE warmup burst: dummy matmuls (deps ready ~1us in) while the
            # first x tile loads; the p-state model needs ~3.4us of early PE
            # activity to reach the full 2.4GHz clock.
            warm_ps = zps.tile([128, 1024], f32, name="warm_ps", tag="z")
            for _ in range(32):
                nc.tensor.matmul(
                    warm_ps[:, 0:128], W_sb, ones[:], skip_group_check=True
                )

            prev = None   # state of tile t-1 awaiting its score/agg tail

            def emit_head(st):
                """e = lrelu(s) (+bias), w = exp(e). First ACT ops of a tile."""
                ns = st["subs"]
                s_ps = st["s_ps"][:, 0:32 * ns]
                e_sb = smallp.tile([128, 64], f32, name="e_sb", tag="e_sb")
                if with_bias:
                    sb = smallp.tile([128, 64], f32, name="sb", tag="sb")
                    nc.vector.tensor_tensor(sb[:, 0:32 * ns], s_ps, b64[:, 0:32 * ns], OP.add)
                    nc.scalar.activation(e_sb[:, 0:32 * ns], sb[:, 0:32 * ns], AF.Prelu, alpha=NEG)
                else:
                    nc.scalar.activation(e_sb[:, 0:32 * ns], s_ps, AF.Prelu, alpha=NEG)
                w_sb = smallp.tile([128, 64], bf16, name="w_sb", tag="w_sb")
                nc.scalar.activation(w_sb[:, 0:32 * ns], e_sb[:, 0:32 * ns], AF.Exp)
                st["w_sb"] = w_sb
                if debug:
                    sd = smallp.tile([128, 64], f32, name="sd", tag="sd")
                    nc.vector.tensor_copy(sd[:, 0:32 * ns], s_ps)
                    nc.sync.dma_start(sdump_ext[st["t"]][:, 0:32 * ns], sd[:, 0:32 * ns])
                    nc.sync.dma_start(wdump_ext[st["t"]][:, 0:32 * ns], w_sb[:, 0:32 * ns])
                    nc.sync.dma_start(hdump_ext[st["t"]], st["h01"])

            def emit_transp(st):
                """8 32x32 transposes: sub s node-quarter j -> aT4_ps rows
                [32j,32j+32), cols [128s+32j, +32). Then one DVE copy to SBUF
                (zeros off-block come along) + gpsimd per-node sums + DMA."""
                w_sb = st["w_sb"]
                for s in range(st["subs"]):
                    for j in range(4):
                        nc.tensor.transpose(
                            aT4_ps[32 * j:32 * j + 32,
                                    128 * s + 32 * j:128 * s + 32 * j + 32],
                            w_sb[32 * j:32 * j + 32, 32 * s:32 * s + 32],
                            ident[32 * j:32 * j + 32, 32 * j:32 * j + 32],
                            tile_position=(32 * j, 32 * j),
                        )
                aT4 = smallp.tile([128, TN], bf16, name="aT4", tag="aT4")
                nc.vector.tensor_copy(
                    aT4[:, 0:128 * st["subs"]], aT4_ps[:, 0:128 * st["subs"]]
                )
                st["aT4"] = aT4
                st["o_ps"] = o_ps_ap

            def emit_agg(st, q):
                """Aggregation for node groups [16q, 16q+16): one 4-column
                matmul per group of 4 nodes; stationary = their label blocks
                stacked [128, 64] fp8 + a 65th all-ones column whose output
                row is the per-node exp-weight sum (softmax denominator)."""
                lab_sb, aT4, o_ps = st["lab_sb"], st["aT4"], st["o_ps"]
                gmax = 32 * st["subs"]
                for g in range(16 * q, min(16 * q + 16, gmax)):
                    # group g = (s, m): the 4 nodes 128s+32j+m (j=0..3), one
                    # per partition block of aT4 -> moving cols stride 32.
                    # Output columns are contiguous [4g, 4g+4); the host
                    # remaps column 128s+4m+j back to node 128s+32j+m.
                    s_g, m = g // 32, g % 32
                    nc.tensor.matmul(
                        o_ps[:, 4 * g:4 * g + 4],
                        lab_sb[:, 65 * g:65 * g + 65],
                        aT4[:, 128 * s_g + m:128 * s_g + m + 97:32],
                    )

            def emit_out(st):
                w = 128 * st["subs"]
                o_sb = outp.tile([L + 1, TN], bf16, name="o_sb", tag="o_sb")
                nc.scalar.activation(o_sb[:, 0:w], st["o_ps"][:, 0:w], AF.Copy)
                nc.sync.dma_start(out_ext[st["t"]][:, 0:w], o_sb[:, 0:w])

            for t in range(nt):
                x_sb = xp.tile([128, K * TN], f8)
                if t == 0:
                    # quarter the first x load so chunk 0's matmuls start
                    # after ~256KB instead of a full 1MB
                    qn = K * TN // 4
                    for qi in range(4):
                        nc.sync.dma_start(
                            x_sb[:, qi * qn:(qi + 1) * qn],
                            x_ext[t][:, qi * qn:(qi + 1) * qn],
                        )
                else:
                    nc.sync.dma_start(x_sb[:], x_ext[t][:])
                lab_sb = labp.tile([128, 64 * 65], f8)
                nc.sync.dma_start(lab_sb[:], lab_ext[t][:])

                h_sb = hp.tile([128, K * TN], bf16)
                s_ps = s_ps_ab[t % 2]
                tsubs = 1 if t == nt - 1 else NSUB

                def emit_scores(c):
                    # score columns for chunk c's k values (ready once relu c
                    # done). For relu-only (DVE) chunks each column is a
                    # 2-matmul group: h-term then the 0.2(Wv)^T x correction.
                    # Keeping the group members ADJACENT matters: a group
                    # split across other matmuls loses the first term.
                    dve = SCHED[c] == "D"
                    for k in range(4 * c, 4 * c + 4):
                        for s in range(tsubs):
                            col = 32 * s + k
                            base = k * TN + s * 128
                            nc.tensor.matmul(
                                s_ps[:, col:col + 1],
                                h_sb[:, base:base + 128],
                                v08 if dve else v10,
                                start=True, stop=not dve,
                            )
                            if dve:
                                nc.tensor.matmul(
                                    s_ps[:, col:col + 1],
                                    x_sb[:, base:base + 128], wv02,
                                    start=False, stop=True,
                                )

                for c in range(NCH):
                    z_ps = zps.tile([128, 1024], f32, name="z_ps", tag="z")
                    nc.tensor.matmul(
                        z_ps[:, 0:512], W_sb, x_sb[:, c * 1024:c * 1024 + 512]
                    )
                    nc.tensor.matmul(
                        z_ps[:, 512:1024], W_sb,
                        x_sb[:, c * 1024 + 512:(c + 1) * 1024],
                    )
                    if t == nt - 1:
                        # padding-only sub-tiles: activate the first 128
                        # nodes per k only (the rest is never read)
                        zv = z_ps[:].rearrange("p (k n) -> p k n", k=4)[:, :, 0:128]
                        hv = h_sb[:, c * 1024:(c + 1) * 1024].rearrange(
                            "p (k n) -> p k n", k=4)[:, :, 0:128]
                    else:
                        zv = z_ps[:]
                        hv = h_sb[:, c * 1024:(c + 1) * 1024]
                    if SCHED[c] == "A":
                        nc.scalar.activation(hv, zv, AF.Prelu, alpha=NEG)
                    else:
                        nc.vector.tensor_scalar_max(hv, zv, 0.0)
                    if c == 0 and prev is not None:
                        # the head chain gates next-tile aT4/agg: make it
                        # look one tile older so the scheduler runs it ASAP
                        with tc.high_priority(offset=600):
                            emit_head(prev)
                    if c == 1 and prev is not None:
                        with tc.high_priority(offset=600):
                            emit_transp(prev)
                    if c in (3, 4, 5, 6) and prev is not None:
                        emit_agg(prev, c - 3)
                    if c >= 3:
                        emit_scores(c - 3)
                    if c == 7 and prev is not None:
                        emit_out(prev)
                for cc in (NCH - 3, NCH - 2, NCH - 1):
                    emit_scores(cc)

                prev = {"t": t, "s_ps": s_ps, "lab_sb": lab_sb, "subs": tsubs,
                        "h01": h_sb[:, 0:2048]}

            # drain the last tile
            emit_head(prev)
            emit_transp(prev)
            for q in range(4):
                emit_agg(prev, q)
            emit_out(prev)
    nc.compile()
    return nc


def shard_x(x, nt=NT, nper=NPER):
    import ml_dtypes

    f8 = ml_dtypes.float8_e3m4
    xs = np.zeros((nt * TN, K, D), f8)
    xs[:nper] = x.astype(f8)
    # [t, n, k, d] -> [t, d, k, n] -> col = k*TN + n
    return np.ascontiguousarray(
        xs.reshape(nt, TN, K, D).transpose(0, 3, 2, 1)
    ).reshape(nt, 128, K * TN)


def shard_lab(lab, nt=NT, nper=NPER):
    import ml_dtypes

    f8 = ml_dtypes.float8_e3m4
    ls = np.zeros((nt * TN, K, L + 1), f8)
    ls[:nper, :, :L] = lab.astype(f8)
    ls[:, :, L] = f8(1.0)   # ones column -> per-node exp-weight sums
    # node n = 128s + 32j + m belongs to group g = 32s + m with quarter j:
    # its labels sit at rows 32j + k of group block [65g, 65g+65)
    l6 = ls.reshape(nt, NSUB, 4, 32, K, L + 1)   # [t, s, j, m, k, l]
    return np.ascontiguousarray(
        l6.transpose(0, 2, 4, 1, 3, 5)           # [t, j, k, s, m, l]
    ).reshape(nt, 128, 64 * 65)


def make_in_maps(inputs):
    import ml_dtypes

    bf16 = ml_dtypes.bfloat16
    x = np.asarray(inputs["para_neighbors"], np.float32)
    lab = np.asarray(inputs["para_nei_labels"], np.float32)
    Wm = np.ascontiguousarray(np.asarray(inputs["linear"], np.float32))
    v = np.ascontiguousarray(np.asarray(inputs["e_vec"], np.float32))
    b = np.asarray(inputs["bias"], np.float32).reshape(K)

    Wb = Wm.astype(bf16).astype(np.float32)
    vb = v.astype(bf16).astype(np.float32)
    wpk = np.zeros((128, 131), bf16)
    wpk[:, 0:128] = Wm.astype(bf16)
    wpk[:, 128] = vb.astype(bf16).reshape(128)
    wpk[:, 129] = (0.8 * vb).astype(bf16).reshape(128)
    # 0.2*(W@v) from the bf16-rounded W/v so the correction matches the PE's z
    wpk[:, 130] = (NEG * (Wb @ vb)).astype(bf16).reshape(128)
    wpk = np.ascontiguousarray(wpk)
    # b64[p, 32s+k] = bias[k] (same for every partition row)
    b64 = np.ascontiguousarray(
        np.tile(np.concatenate([b, b])[None, :], (128, 1))
    ).astype(np.float32)

    in_maps = []
    for i in range(NCORES):
        xf = shard_x(x[i * NPER:(i + 1) * NPER])
        lf = shard_lab(lab[i * NPER:(i + 1) * NPER])
        in_maps.append({"x": xf, "lab": lf, "wpk": wpk, "b64": b64})
    return in_maps


def unshard_output(res_i):
    # out[t, l, c]: column c = 128s + 4m + j holds node n = 128s + 32j + m
    # of tile t; row L = exp-weight sum. Softmax normalization happens here.
    o = np.asarray(res_i["out"]).astype(np.float32)      # [nt, L+1, TN]
    c = np.arange(TN)
    node_of_c = 128 * (c // 128) + 32 * (c % 4) + (c % 128) // 4
    inv = np.empty(TN, np.int64)
    inv[node_of_c] = c
    o = o[:, :, inv]                                     # column c' = node c'
    raw = o[:, :L].transpose(0, 2, 1).reshape(NT * TN, L)
    s = o[:, L].reshape(NT * TN)
    return (raw[:NPER] / s[:NPER, None]).astype(np.float32)


def kernel(para_neighbors, para_nei_labels, linear, e_vec, bias):
    from concourse.bass_utils import run_bass_kernel_spmd

    global LAST_RESULT
    with_bias = bool(np.any(np.asarray(bias)))
    key = ("nc", with_bias)
    if key not in _cache:
        _cache[key] = build(with_bias)
        _cache["nc"] = _cache[key]
    nc = _cache[key]

    in_maps = make_in_maps({
        "para_neighbors": para_neighbors, "para_nei_labels": para_nei_labels,
        "linear": linear, "e_vec": e_vec, "bias": bias,
    })
    res = run_bass_kernel_spmd(nc, in_maps, core_ids=list(range(NCORES)))
    LAST_RESULT = res
    outs = [unshard_output(res.results[i]) for i in range(NCORES)]
    return np.ascontiguousarray(np.concatenate(outs, axis=0))
